# revision 33
# baseline (speedup 1.0000x reference)
"""Trainium2 Bass kernel for BlockUncertaintyTracker (segment_reduce), v2.

Per 4x4 block of a [16,1,2048,2048] f32 batch: block mean and
0.9-quantile (= 0.5*(2nd+3rd largest of 16)), averaged over batch, EMA'd,
ratio broadcast back. Spatial sharding over H across 8 cores (64 block
rows per core, all 16 batches per core, no collectives).

v2 layout trick: the f32->f16 cast on ScalarE uses a stride-4 source so
each cast writes one intra-block column as a contiguous 512-wide segment
(D_r = [c0|c1|c2|c3]). Every later op is then step-1 contiguous:
  - L1 pair max/min (GpSimd, 4 ops [128,2048])
  - L2 sorted-3-of-4 per column (DVE, 5 ops [128,2048])
  - L3 merge col pairs (0,1),(2,3) (DVE, 7 dual-segment ops [128,2x512])
  - L4 final merge -> 2nd,3rd (DVE, 7 ops [128,512])
  - mean: 16 f16 matmuls per group on D_r segments (PE, exact fp32 acc)
Output: only the per-block u map [64,512] per core (host broadcasts).
"""

import os

import numpy as np

# ---- problem constants (hardcoded; kernel.py must be self-contained) ----
B = 16          # batch
H = 2048
W = 2048
BS = 4          # block size
NCORES = 8
HS = H // NCORES            # 256 rows per core
NBH = HS // BS              # 64 block rows per core
NBW = W // BS               # 512 block cols
ROWS = B * HS               # 4096 rows in a per-core slab
NGROUPS = 8                 # groups per core; each = 2 batches x 256 rows
GB = B // NGROUPS           # 2 batches per group
DECAY = 0.99
ALPHA = 0.1
EPS = 1e-5
C_MEAN = (1.0 - DECAY) / (BS * BS * B)    # fold mean-over-16-elems and batch
C_QUANT = (1.0 - DECAY) * 0.5 / B         # fold 0.5*(m2+m3) and batch mean

_CACHE = {}


def _split_multi_waits(nc):
    """This walrus build encodes at most ONE sync wait per instruction.
    Tile attaches several. Hoist excess waits onto same-engine NOPs placed
    immediately before the owning instruction (same engine stream => same
    semantics)."""
    import concourse.mybir as mybir

    plans = []  # (inst_name, extra_waits)
    for f in nc.m.functions:
        for bb in f.blocks:
            for inst in bb.instructions:
                si = getattr(inst, "sync_info", None)
                waits = list(si.on_wait) if (si and si.on_wait) else []
                if len(waits) > 1:
                    si.on_wait = [waits[-1]]
                    plans.append((inst.name, waits[:-1]))

    if not plans:
        return

    nop_for = {}
    stray = set()
    for iname, extra in plans:
        nops = []
        for w in extra:
            nop = nc.engines[nc.inst_map[iname].engine].nop(nofuse=True).ins
            nop.sync_info = mybir.SyncInfo(on_wait=[w], on_update=[])
            nops.append(nop)
            stray.add(nop.name)
        nop_for[iname] = nops

    for f in nc.m.functions:
        for bb in f.blocks:
            out = []
            changed = False
            for inst in bb.instructions:
                if inst.name in stray:
                    changed = True
                    continue
                if inst.name in nop_for:
                    out.extend(nop_for[inst.name])
                    changed = True
                out.append(inst)
            if changed:
                bb.instructions = out


def _build():
    """Builds the single-core Bass program (SPMD across 8 cores)."""
    from contextlib import ExitStack

    import concourse.bass as bass
    import concourse.mybir as mybir
    import concourse.tile as tile

    f32 = mybir.dt.float32
    f16 = mybir.dt.float16
    MAX = mybir.AluOpType.max
    MIN = mybir.AluOpType.min
    MULT = mybir.AluOpType.mult
    ADD = mybir.AluOpType.add

    nc = bass.Bass("TRN2", target_bir_lowering=False, debug=False)

    f32r = mybir.dt.float32r
    x = nc.dram_tensor("x", [ROWS, W], f32r, kind="ExternalInput").ap()
    ee = nc.dram_tensor("ee", [NBH, NBW], f32, kind="ExternalInput").ap()
    eq = nc.dram_tensor("eq", [NBH, NBW], f32, kind="ExternalInput").ap()
    # ones2[p, m] = (p % 64 == m // 2): batch-pair fold + row duplication
    ones2 = nc.dram_tensor("ones2", [128, 128], f32, kind="ExternalInput").ap()
    ident = nc.dram_tensor("ident", [128, 128], f32r, kind="ExternalInput").ap()
    # per-block uncertainty map only; host broadcasts to [B, H, W]
    y = nc.dram_tensor("y", [NBH, NBW], f32, kind="ExternalOutput").ap()

    # input: row = ((g*2 + b2)*64 + i)*4 + r; per (g, r): [128=(b2,i), 2048]
    xr = x.rearrange("(g b2 i r) w -> g r (b2 i) w", g=NGROUPS, b2=GB, i=NBH, r=BS)

    with tile.TileContext(nc) as tc, ExitStack() as ctx:
        pool = ctx.enter_context(tc.tile_pool(name="work", bufs=1))
        ppool = ctx.enter_context(tc.tile_pool(name="acc", bufs=1, space="PSUM"))

        psum_s = ppool.tile([128, NBW], f32, tag="ps")
        psum_q = ppool.tile([128, NBW], f32, tag="pq")

        ones_sb = pool.tile([128, 128], f32, tag="ones")
        nc.sync.dma_start(ones_sb[:, :], ones2)
        ones16_sb = pool.tile([128, 128], f16, tag="ones16")
        nc.scalar.copy(ones16_sb[:, :], ones_sb[:, :])
        ident_sb = pool.tile([128, 128], f32r, tag="ident")
        nc.sync.dma_start(ident_sb[:, :], ident)

        def vtt(dst, a, bb, op):
            nc.vector.tensor_tensor(dst, a, bb, op)

        rts_g = {}
        dts_g = {}
        l1_g = {}
        l2_g = {}
        l3_g = {}
        l4_g = {}
        nmm = [0]

        def emit_load(g):
            rts = []
            for r in range(BS):
                rt = pool.tile([128, W], f32r, tag=f"r{r}", bufs=2, name=f"rt{r}_{g}")
                nc.sync.dma_start(rt[:, :], xr[g, r])
                rts.append(rt)
            rts_g[g] = rts

        def emit_cast_tile(g, r):
            # column-split via PE: identity matmul reads rt strided (the
            # 16B-pitch penalty lands on the idle tensor engine) into a
            # rotating PSUM bank; ACT then does a cheap contiguous
            # PSUM->SBUF f16 cast. D_r = [c0|c1|c2|c3] as before.
            dt = pool.tile([128, W], f16, tag=f"d{r}", bufs=2, name=f"dt{r}_{g}")
            dv = dt.rearrange("p (c j) -> p c j", c=BS)
            sv = rts_g[g][r].rearrange("p (j c) -> p j c", c=BS)
            # paired segment order [c0|c2|c1|c3]: L3's two merges read
            # contiguous 1024-wide halves (lo = c0,c2 vs hi = c1,c3)
            for seg, c in enumerate((0, 2, 1, 3)):
                pc = ppool.tile([128, NBW], f32, tag="pc", bufs=6,
                                name=f"pc_{g}_{r}_{c}")
                nc.tensor.matmul(
                    pc[:, :], lhsT=ident_sb[:, :], rhs=sv[:, :, c],
                    start=True, stop=True,
                )
                nc.scalar.copy(dv[:, seg, :], pc[:, :])
            dts_g.setdefault(g, []).append(dt)

        def emit_sum(g, tiles):
            # exact block sums: 16 f16 matmuls on contiguous 512-segments.
            # `tiles` per-position sum must equal D0+D1+D2+D3 (for the last
            # group we pass L1's max/min outputs: max+min == sum).
            for r in range(BS):
                dv = tiles[r].rearrange("p (c j) -> p c j", c=BS)
                for c in range(BS):
                    k = g * 16 + r * BS + c
                    nc.tensor.matmul(
                        psum_s[:, :], lhsT=ones16_sb[:, :], rhs=dv[:, c, :],
                        start=(k == 0), stop=(k == NGROUPS * 16 - 1),
                    )
                    nmm[0] += 1

        def emit_l1_pair(g, pair):
            # one tile-pair's max/min (pair 0 = (d0,d1), pair 1 = (d2,d3))
            a, bb = dts_g[g][2 * pair], dts_g[g][2 * pair + 1]
            for idx, op in ((2 * pair, MAX), (2 * pair + 1, MIN)):
                t = pool.tile([128, W], f16, tag="l1", bufs=8, name=f"l1_{idx}_{g}")
                vtt(t[:, :], a[:, :], bb[:, :], op)
                l1_g.setdefault(g, []).append(t)

        def emit_l2(g):
            A, Bm, C, E = l1_g[g]  # A=max01 Bm=min01 C=max23 E=min23
            # L2: per-column sorted top-3 of 4 (full width, column-split)
            r1 = pool.tile([128, W], f16, tag="l2m", bufs=6, name=f"r1_{g}")
            vtt(r1[:, :], A[:, :], C[:, :], MAX)
            xx = pool.tile([128, W], f16, tag="l2t", bufs=2, name=f"xx_{g}")
            vtt(xx[:, :], A[:, :], C[:, :], MIN)
            yy = pool.tile([128, W], f16, tag="l2t", bufs=2, name=f"yy_{g}")
            vtt(yy[:, :], Bm[:, :], E[:, :], MAX)
            r2 = pool.tile([128, W], f16, tag="l2m", bufs=6, name=f"r2_{g}")
            vtt(r2[:, :], xx[:, :], yy[:, :], MAX)
            r3 = pool.tile([128, W], f16, tag="l2m", bufs=6, name=f"r3_{g}")
            vtt(r3[:, :], xx[:, :], yy[:, :], MIN)
            l2_g[g] = (r1, r2, r3)

        def emit_l3(g):
            r1, r2, r3 = l2_g[g]
            # L3: paired layout [c0|c2|c1|c3] makes both column merges one
            # contiguous 1024-wide op: lo half = (c0,c2), hi half = (c1,c3)
            # -> the two merges (c0,c1) and (c2,c3) line up elementwise.
            HW2 = W // 2

            def mk(name):
                return pool.tile([128, HW2], f16, tag="l3", bufs=7,
                                 name=f"{name}_{g}")

            def lo(t):
                return t[:, 0:HW2]

            def hi(t):
                return t[:, HW2:W]

            s1, x2, c2, s2, mn, m3, s3 = (
                mk("s1"), mk("x2"), mk("c2"), mk("s2"), mk("mn"), mk("m3"),
                mk("s3"),
            )
            vtt(s1[:, :], lo(r1), hi(r1), MAX)   # pair rank1
            vtt(x2[:, :], lo(r1), hi(r1), MIN)
            vtt(c2[:, :], lo(r2), hi(r2), MAX)
            vtt(s2[:, :], x2[:, :], c2[:, :], MAX)  # pair rank2
            vtt(mn[:, :], x2[:, :], c2[:, :], MIN)
            vtt(m3[:, :], lo(r3), hi(r3), MAX)
            vtt(s3[:, :], mn[:, :], m3[:, :], MAX)  # pair rank3
            l3_g[g] = (s1, s2, s3)

        def emit_l4(g):
            s1, s2, s3 = l3_g[g]
            a1, b1 = s1[:, 0:NBW], s1[:, NBW : 2 * NBW]
            a2, b2 = s2[:, 0:NBW], s2[:, NBW : 2 * NBW]
            a3, b3 = s3[:, 0:NBW], s3[:, NBW : 2 * NBW]

            def op4(name, ina, inb, op):
                t = pool.tile([128, NBW], f16, tag="l4", bufs=7, name=f"{name}_{g}")
                vtt(t[:, :], ina, inb, op)
                return t

            x4 = op4("x4", a1, b1, MIN)
            c4 = op4("c4", a2, b2, MAX)
            r2f = op4("r2f", x4[:, :], c4[:, :], MAX)
            mn4 = op4("mn4", x4[:, :], c4[:, :], MIN)
            m34 = op4("m34", a3, b3, MAX)
            r3f = op4("r3f", mn4[:, :], m34[:, :], MAX)
            l4_g[g] = (r2f, r3f)

        def emit_qmm(g):
            # qs = r2f + r3f folded into PSUM accumulation: two matmuls
            r2f, r3f = l4_g[g]
            nc.tensor.matmul(
                psum_q[:, :], lhsT=ones16_sb[:, :], rhs=r2f[:, :],
                start=(g == 0), stop=False,
            )
            nc.tensor.matmul(
                psum_q[:, :], lhsT=ones16_sb[:, :], rhs=r3f[:, :],
                start=False, stop=(g == NGROUPS - 1),
            )

        # EMA input DMAs + pre-scale early (independent of the group stream)
        ee_sb = pool.tile([128, NBW], f32, tag="tail", bufs=8, name="ee_sb")
        nc.sync.dma_start(ee_sb[:, :], ee.unsqueeze(1).broadcast_to((NBH, 2, NBW)))
        eq_sb = pool.tile([128, NBW], f32, tag="tail", bufs=8, name="eq_sb")
        nc.sync.dma_start(eq_sb[:, :], eq.unsqueeze(1).broadcast_to((NBH, 2, NBW)))
        ee2 = pool.tile([128, NBW], f32, tag="tail", bufs=8, name="ee2")
        nc.scalar.activation(
            ee2[:, :], ee_sb[:, :], mybir.ActivationFunctionType.Copy,
            bias=EPS, scale=DECAY,
        )
        eq2 = pool.tile([128, NBW], f32, tag="tail", bufs=8, name="eq2")
        nc.scalar.activation(
            eq2[:, :], eq_sb[:, :], mybir.ActivationFunctionType.Copy,
            bias=0.0, scale=DECAY,
        )

        LAST = NGROUPS - 1
        for g in range(NGROUPS):
            emit_load(g)
            # interleave casts and L1 pairs so DVE starts ~halfway into the
            # cast stream; sums/qmm of the previous group are emitted AFTER
            # this group's PE copies so the in-order PE stream never delays
            # the cast chain feeding DVE
            emit_cast_tile(g, 0)
            emit_cast_tile(g, 1)
            emit_l1_pair(g, 0)
            emit_cast_tile(g, 2)
            emit_cast_tile(g, 3)
            if g >= 1:
                emit_sum(g - 1, dts_g[g - 1])
                emit_qmm(g - 1)
            emit_l1_pair(g, 1)
            emit_l2(g)
            if g == LAST:
                emit_sum(g, dts_g[g])
            if g != LAST:
                emit_l3(g)
                emit_l4(g)

        # den/reciprocal depend only on psum_s (complete after last emit_sum):
        # issue them on DVE before the last group's L3/L4 to shorten the tail
        den = pool.tile([128, NBW], f32, tag="tail", bufs=8, name="den")
        nc.vector.scalar_tensor_tensor(
            den[:, :], psum_s[:, :], C_MEAN, ee2[:, :], op0=MULT, op1=ADD
        )
        rec = pool.tile([128, NBW], f32, tag="tail", bufs=8, name="rec")
        nc.vector.reciprocal(rec[:, :], den[:, :])

        emit_l3(LAST)
        emit_l4(LAST)
        emit_qmm(LAST)

        num = pool.tile([128, NBW], f32, tag="tail", bufs=8, name="num")
        nc.vector.scalar_tensor_tensor(
            num[:, :], psum_q[:, :], C_QUANT, eq2[:, :], op0=MULT, op1=ADD
        )
        u = pool.tile([128, NBW], f32, tag="tail", bufs=8, name="u")
        nc.vector.tensor_tensor(u[:, :], num[:, :], rec[:, :], MULT)

        # single small write: u[2i, j] == u[2i+1, j] == per-block value
        nc.sync.dma_start(y, u[0:128:2, :])

    _split_multi_waits(nc)
    return nc


def _get_nc():
    if "nc" not in _CACHE:
        _CACHE["nc"] = _build()
    return _CACHE["nc"]


def kernel(current_errors, ema_errors, ema_quantile):
    from concourse.bass_utils import run_bass_kernel_spmd

    x = np.asarray(current_errors, dtype=np.float32).reshape(B, H, W)
    ee = np.asarray(ema_errors, dtype=np.float32).reshape(H // BS, W // BS)
    eq = np.asarray(ema_quantile, dtype=np.float32).reshape(H // BS, W // BS)

    # ones2[p, m] == 1 iff p % 64 == m // 2
    ones2 = np.zeros((128, 128), dtype=np.float32)
    p = np.arange(128)
    ones2[p, (p % NBH) * 2] = 1.0
    ones2[p, (p % NBH) * 2 + 1] = 1.0
    ident = np.eye(128, dtype=np.float32)

    in_maps = []
    for k in range(NCORES):
        xs = np.ascontiguousarray(x[:, k * HS : (k + 1) * HS, :]).reshape(ROWS, W)
        ees = np.ascontiguousarray(ee[k * NBH : (k + 1) * NBH, :])
        eqs = np.ascontiguousarray(eq[k * NBH : (k + 1) * NBH, :])
        in_maps.append(
            {"x": xs, "ee": ees, "eq": eqs, "ones2": ones2, "ident": ident}
        )

    nc = _get_nc()
    trace = bool(int(os.environ.get("KERNEL_TRACE", "0")))
    try:
        res = run_bass_kernel_spmd(
            nc, in_maps, core_ids=list(range(NCORES)), trace=trace
        )
    except Exception:
        # transient device state (e.g. NRT_EXEC_UNIT_UNRECOVERABLE) — retry once
        res = run_bass_kernel_spmd(
            nc, in_maps, core_ids=list(range(NCORES)), trace=trace
        )
    _CACHE["last_results"] = res

    # gather/unshard: stack per-core block maps, broadcast back onto the grid
    ub = np.concatenate(
        [res.results[k]["y"].reshape(NBH, NBW) for k in range(NCORES)], axis=0
    )  # [512, 512] per-block values
    plane = np.repeat(np.repeat(ub, BS, axis=0), BS, axis=1)  # [H, W]
    out = np.empty((B, 1, H, W), dtype=np.float32)
    out[:] = plane[None, None]
    return out


# revision 34
# speedup vs baseline: 1.1769x; 1.1769x over previous
"""Trainium2 Bass kernel for BlockUncertaintyTracker (segment_reduce), v7.

Per 4x4 block of a [16,1,2048,2048] f32 batch: block mean and 0.9-quantile
(= 0.5*(2nd+3rd largest of 16)), averaged over batch, EMA'd, ratio
broadcast back. Spatial sharding over H across 8 cores (64 block rows per
core, all 16 batches per core, no collectives). 8 groups/core of 2 batches
x 256 rows, each as 4 row-phase tiles [128=(b2,i), 2048] f32.

Key engine assignments (measured costs drove these):
  - Deinterleave+downcast: PE identity matmuls (f32r, strided rhs -- the
    16B-pitch SBUF read penalty lands on the underused tensor engine) into
    rotating PSUM banks, then cheap contiguous PSUM->SBUF f16 casts on
    ACT. Paired segment order D_r = [c0|c2|c1|c3].
  - Selection network on DVE, all step-1 contiguous f16 (2x mode):
    L1 pair max/min (4 ops [128,2048]), L2 sorted-3-of-4 per column
    (5 ops [128,2048]), L3 both column-pair merges as one contiguous op
    (7 ops [128,1024]), L4 final merge -> 2nd,3rd (6 ops [128,512]).
    NOTE: multi-dim 2-byte APs on DVE 2x mode give wrong results on HW
    (diverges from CoreSim); only plain 1-D step-1 slices are used.
  - Mean: 16 f16 matmuls/group on D segments (fp32 PSUM accumulate);
    quantile batch-fold: 2 matmuls/group on r2f/r3f (folds the + too).
  - Software pipelining: sums/qmm of group g-1 are emitted after group
    g's PE copies so the in-order PE stream never stalls the cast chain;
    EMA prescale on ACT; den/reciprocal issued before the last L3/L4.
Output: only the per-block u map [64,512] f32 per core; the host
broadcast to [B,1,H,W] is the unshard step (u is batch-independent).
"""
import os

import numpy as np

# ---- problem constants (hardcoded; kernel.py must be self-contained) ----
B = 16          # batch
H = 2048
W = 2048
BS = 4          # block size
NCORES = 8
HS = H // NCORES            # 256 rows per core
NBH = HS // BS              # 64 block rows per core
NBW = W // BS               # 512 block cols
ROWS = B * HS               # 4096 rows in a per-core slab
NGROUPS = 8                 # groups per core; each = 2 batches x 256 rows
GB = B // NGROUPS           # 2 batches per group
DECAY = 0.99
ALPHA = 0.1
EPS = 1e-5
C_MEAN = (1.0 - DECAY) / (BS * BS * B)    # fold mean-over-16-elems and batch
C_QUANT = (1.0 - DECAY) * 0.5 / B         # fold 0.5*(m2+m3) and batch mean

_CACHE = {}


def _split_multi_waits(nc):
    """This walrus build encodes at most ONE sync wait per instruction.
    Tile attaches several. Hoist excess waits onto same-engine NOPs placed
    immediately before the owning instruction (same engine stream => same
    semantics)."""
    import concourse.mybir as mybir

    plans = []  # (inst_name, extra_waits)
    for f in nc.m.functions:
        for bb in f.blocks:
            for inst in bb.instructions:
                si = getattr(inst, "sync_info", None)
                waits = list(si.on_wait) if (si and si.on_wait) else []
                if len(waits) > 1:
                    si.on_wait = [waits[-1]]
                    plans.append((inst.name, waits[:-1]))

    if not plans:
        return

    nop_for = {}
    stray = set()
    for iname, extra in plans:
        nops = []
        for w in extra:
            nop = nc.engines[nc.inst_map[iname].engine].nop(nofuse=True).ins
            nop.sync_info = mybir.SyncInfo(on_wait=[w], on_update=[])
            nops.append(nop)
            stray.add(nop.name)
        nop_for[iname] = nops

    for f in nc.m.functions:
        for bb in f.blocks:
            out = []
            changed = False
            for inst in bb.instructions:
                if inst.name in stray:
                    changed = True
                    continue
                if inst.name in nop_for:
                    out.extend(nop_for[inst.name])
                    changed = True
                out.append(inst)
            if changed:
                bb.instructions = out


def _build():
    """Builds the single-core Bass program (SPMD across 8 cores)."""
    from contextlib import ExitStack

    import concourse.bass as bass
    import concourse.mybir as mybir
    import concourse.tile as tile

    f32 = mybir.dt.float32
    f16 = mybir.dt.float16
    MAX = mybir.AluOpType.max
    MIN = mybir.AluOpType.min
    MULT = mybir.AluOpType.mult
    ADD = mybir.AluOpType.add

    nc = bass.Bass("TRN2", target_bir_lowering=False, debug=False)

    f32r = mybir.dt.float32r
    x = nc.dram_tensor("x", [ROWS, W], f32r, kind="ExternalInput").ap()
    ee = nc.dram_tensor("ee", [NBH, NBW], f32, kind="ExternalInput").ap()
    eq = nc.dram_tensor("eq", [NBH, NBW], f32, kind="ExternalInput").ap()
    # ones2[p, m] = (p % 64 == m // 2): batch-pair fold + row duplication
    ones2 = nc.dram_tensor("ones2", [128, 128], f32, kind="ExternalInput").ap()
    ident = nc.dram_tensor("ident", [128, 128], f32r, kind="ExternalInput").ap()
    # per-block uncertainty map only; host broadcasts to [B, H, W]
    y = nc.dram_tensor("y", [NBH, NBW], f32, kind="ExternalOutput").ap()

    # input: row = ((g*2 + b2)*64 + i)*4 + r; per (g, r): [128=(b2,i), 2048]
    xr = x.rearrange("(g b2 i r) w -> g r (b2 i) w", g=NGROUPS, b2=GB, i=NBH, r=BS)

    with tile.TileContext(nc) as tc, ExitStack() as ctx:
        pool = ctx.enter_context(tc.tile_pool(name="work", bufs=1))
        ppool = ctx.enter_context(tc.tile_pool(name="acc", bufs=1, space="PSUM"))

        psum_s = ppool.tile([128, NBW], f32, tag="ps")
        psum_q = ppool.tile([128, NBW], f32, tag="pq")

        ones_sb = pool.tile([128, 128], f32, tag="ones")
        nc.sync.dma_start(ones_sb[:, :], ones2)
        ones16_sb = pool.tile([128, 128], f16, tag="ones16")
        nc.scalar.copy(ones16_sb[:, :], ones_sb[:, :])
        ident_sb = pool.tile([128, 128], f32r, tag="ident")
        nc.sync.dma_start(ident_sb[:, :], ident)

        def vtt(dst, a, bb, op):
            nc.vector.tensor_tensor(dst, a, bb, op)

        rts_g = {}
        dts_g = {}
        l1_g = {}
        l2_g = {}
        l3_g = {}
        l4_g = {}
        nmm = [0]

        def emit_load(g):
            rts = []
            for r in range(BS):
                rt = pool.tile([128, W], f32r, tag=f"r{r}", bufs=2, name=f"rt{r}_{g}")
                nc.sync.dma_start(rt[:, :], xr[g, r])
                rts.append(rt)
            rts_g[g] = rts

        def emit_cast_tile(g, r):
            # column-split via PE: identity matmul reads rt strided (the
            # 16B-pitch penalty lands on the idle tensor engine) into a
            # rotating PSUM bank; ACT then does a cheap contiguous
            # PSUM->SBUF f16 cast. D_r = [c0|c1|c2|c3] as before.
            dt = pool.tile([128, W], f16, tag=f"d{r}", bufs=2, name=f"dt{r}_{g}")
            dv = dt.rearrange("p (c j) -> p c j", c=BS)
            sv = rts_g[g][r].rearrange("p (j c) -> p j c", c=BS)
            # paired segment order [c0|c2|c1|c3]: L3's two merges read
            # contiguous 1024-wide halves (lo = c0,c2 vs hi = c1,c3)
            for seg, c in enumerate((0, 2, 1, 3)):
                pc = ppool.tile([128, NBW], f32, tag="pc", bufs=6,
                                name=f"pc_{g}_{r}_{c}")
                nc.tensor.matmul(
                    pc[:, :], lhsT=ident_sb[:, :], rhs=sv[:, :, c],
                    start=True, stop=True,
                )
                nc.scalar.copy(dv[:, seg, :], pc[:, :])
            dts_g.setdefault(g, []).append(dt)

        def emit_sum(g, tiles):
            # exact block sums: 16 f16 matmuls on contiguous 512-segments.
            # `tiles` per-position sum must equal D0+D1+D2+D3 (for the last
            # group we pass L1's max/min outputs: max+min == sum).
            for r in range(BS):
                dv = tiles[r].rearrange("p (c j) -> p c j", c=BS)
                for c in range(BS):
                    k = g * 16 + r * BS + c
                    nc.tensor.matmul(
                        psum_s[:, :], lhsT=ones16_sb[:, :], rhs=dv[:, c, :],
                        start=(k == 0), stop=(k == NGROUPS * 16 - 1),
                    )
                    nmm[0] += 1

        def emit_l1_pair(g, pair):
            # one tile-pair's max/min (pair 0 = (d0,d1), pair 1 = (d2,d3))
            a, bb = dts_g[g][2 * pair], dts_g[g][2 * pair + 1]
            for idx, op in ((2 * pair, MAX), (2 * pair + 1, MIN)):
                t = pool.tile([128, W], f16, tag="l1", bufs=8, name=f"l1_{idx}_{g}")
                vtt(t[:, :], a[:, :], bb[:, :], op)
                l1_g.setdefault(g, []).append(t)

        def emit_l2(g):
            A, Bm, C, E = l1_g[g]  # A=max01 Bm=min01 C=max23 E=min23
            # L2: per-column sorted top-3 of 4 (full width, column-split)
            r1 = pool.tile([128, W], f16, tag="l2m", bufs=6, name=f"r1_{g}")
            vtt(r1[:, :], A[:, :], C[:, :], MAX)
            xx = pool.tile([128, W], f16, tag="l2t", bufs=2, name=f"xx_{g}")
            vtt(xx[:, :], A[:, :], C[:, :], MIN)
            yy = pool.tile([128, W], f16, tag="l2t", bufs=2, name=f"yy_{g}")
            vtt(yy[:, :], Bm[:, :], E[:, :], MAX)
            r2 = pool.tile([128, W], f16, tag="l2m", bufs=6, name=f"r2_{g}")
            vtt(r2[:, :], xx[:, :], yy[:, :], MAX)
            r3 = pool.tile([128, W], f16, tag="l2m", bufs=6, name=f"r3_{g}")
            vtt(r3[:, :], xx[:, :], yy[:, :], MIN)
            l2_g[g] = (r1, r2, r3)

        def emit_l3(g):
            r1, r2, r3 = l2_g[g]
            # L3: paired layout [c0|c2|c1|c3] makes both column merges one
            # contiguous 1024-wide op: lo half = (c0,c2), hi half = (c1,c3)
            # -> the two merges (c0,c1) and (c2,c3) line up elementwise.
            HW2 = W // 2

            def mk(name):
                return pool.tile([128, HW2], f16, tag="l3", bufs=7,
                                 name=f"{name}_{g}")

            def lo(t):
                return t[:, 0:HW2]

            def hi(t):
                return t[:, HW2:W]

            s1, x2, c2, s2, mn, m3, s3 = (
                mk("s1"), mk("x2"), mk("c2"), mk("s2"), mk("mn"), mk("m3"),
                mk("s3"),
            )
            vtt(s1[:, :], lo(r1), hi(r1), MAX)   # pair rank1
            vtt(x2[:, :], lo(r1), hi(r1), MIN)
            vtt(c2[:, :], lo(r2), hi(r2), MAX)
            vtt(s2[:, :], x2[:, :], c2[:, :], MAX)  # pair rank2
            vtt(mn[:, :], x2[:, :], c2[:, :], MIN)
            vtt(m3[:, :], lo(r3), hi(r3), MAX)
            vtt(s3[:, :], mn[:, :], m3[:, :], MAX)  # pair rank3
            l3_g[g] = (s1, s2, s3)

        def emit_l4(g):
            s1, s2, s3 = l3_g[g]
            a1, b1 = s1[:, 0:NBW], s1[:, NBW : 2 * NBW]
            a2, b2 = s2[:, 0:NBW], s2[:, NBW : 2 * NBW]
            a3, b3 = s3[:, 0:NBW], s3[:, NBW : 2 * NBW]

            def op4(name, ina, inb, op):
                t = pool.tile([128, NBW], f16, tag="l4", bufs=7, name=f"{name}_{g}")
                vtt(t[:, :], ina, inb, op)
                return t

            x4 = op4("x4", a1, b1, MIN)
            c4 = op4("c4", a2, b2, MAX)
            r2f = op4("r2f", x4[:, :], c4[:, :], MAX)
            mn4 = op4("mn4", x4[:, :], c4[:, :], MIN)
            m34 = op4("m34", a3, b3, MAX)
            r3f = op4("r3f", mn4[:, :], m34[:, :], MAX)
            l4_g[g] = (r2f, r3f)

        def emit_qmm(g):
            # qs = r2f + r3f folded into PSUM accumulation: two matmuls
            r2f, r3f = l4_g[g]
            nc.tensor.matmul(
                psum_q[:, :], lhsT=ones16_sb[:, :], rhs=r2f[:, :],
                start=(g == 0), stop=False,
            )
            nc.tensor.matmul(
                psum_q[:, :], lhsT=ones16_sb[:, :], rhs=r3f[:, :],
                start=False, stop=(g == NGROUPS - 1),
            )

        # EMA input DMAs + pre-scale early (independent of the group stream)
        ee_sb = pool.tile([128, NBW], f32, tag="tail", bufs=8, name="ee_sb")
        nc.sync.dma_start(ee_sb[:, :], ee.unsqueeze(1).broadcast_to((NBH, 2, NBW)))
        eq_sb = pool.tile([128, NBW], f32, tag="tail", bufs=8, name="eq_sb")
        nc.sync.dma_start(eq_sb[:, :], eq.unsqueeze(1).broadcast_to((NBH, 2, NBW)))
        ee2 = pool.tile([128, NBW], f32, tag="tail", bufs=8, name="ee2")
        nc.scalar.activation(
            ee2[:, :], ee_sb[:, :], mybir.ActivationFunctionType.Copy,
            bias=EPS, scale=DECAY,
        )
        eq2 = pool.tile([128, NBW], f32, tag="tail", bufs=8, name="eq2")
        nc.scalar.activation(
            eq2[:, :], eq_sb[:, :], mybir.ActivationFunctionType.Copy,
            bias=0.0, scale=DECAY,
        )

        LAST = NGROUPS - 1
        for g in range(NGROUPS):
            emit_load(g)
            # interleave casts and L1 pairs so DVE starts ~halfway into the
            # cast stream; sums/qmm of the previous group are emitted AFTER
            # this group's PE copies so the in-order PE stream never delays
            # the cast chain feeding DVE
            emit_cast_tile(g, 0)
            emit_cast_tile(g, 1)
            emit_l1_pair(g, 0)
            emit_cast_tile(g, 2)
            emit_cast_tile(g, 3)
            if g >= 1:
                emit_sum(g - 1, dts_g[g - 1])
                emit_qmm(g - 1)
            emit_l1_pair(g, 1)
            emit_l2(g)
            if g == LAST:
                emit_sum(g, dts_g[g])
            if g != LAST:
                emit_l3(g)
                emit_l4(g)

        # den/reciprocal depend only on psum_s (complete after last emit_sum):
        # issue them on DVE before the last group's L3/L4 to shorten the tail
        den = pool.tile([128, NBW], f32, tag="tail", bufs=8, name="den")
        nc.vector.scalar_tensor_tensor(
            den[:, :], psum_s[:, :], C_MEAN, ee2[:, :], op0=MULT, op1=ADD
        )
        rec = pool.tile([128, NBW], f32, tag="tail", bufs=8, name="rec")
        nc.vector.reciprocal(rec[:, :], den[:, :])

        emit_l3(LAST)
        emit_l4(LAST)
        emit_qmm(LAST)

        num = pool.tile([128, NBW], f32, tag="tail", bufs=8, name="num")
        nc.vector.scalar_tensor_tensor(
            num[:, :], psum_q[:, :], C_QUANT, eq2[:, :], op0=MULT, op1=ADD
        )
        u = pool.tile([128, NBW], f32, tag="tail", bufs=8, name="u")
        nc.vector.tensor_tensor(u[:, :], num[:, :], rec[:, :], MULT)

        # single small write: u[2i, j] == u[2i+1, j] == per-block value
        nc.sync.dma_start(y, u[0:128:2, :])

    _split_multi_waits(nc)
    return nc


def _get_nc():
    if "nc" not in _CACHE:
        _CACHE["nc"] = _build()
    return _CACHE["nc"]


def kernel(current_errors, ema_errors, ema_quantile):
    from concourse.bass_utils import run_bass_kernel_spmd

    x = np.asarray(current_errors, dtype=np.float32).reshape(B, H, W)
    ee = np.asarray(ema_errors, dtype=np.float32).reshape(H // BS, W // BS)
    eq = np.asarray(ema_quantile, dtype=np.float32).reshape(H // BS, W // BS)

    # ones2[p, m] == 1 iff p % 64 == m // 2
    ones2 = np.zeros((128, 128), dtype=np.float32)
    p = np.arange(128)
    ones2[p, (p % NBH) * 2] = 1.0
    ones2[p, (p % NBH) * 2 + 1] = 1.0
    ident = np.eye(128, dtype=np.float32)

    in_maps = []
    for k in range(NCORES):
        xs = np.ascontiguousarray(x[:, k * HS : (k + 1) * HS, :]).reshape(ROWS, W)
        ees = np.ascontiguousarray(ee[k * NBH : (k + 1) * NBH, :])
        eqs = np.ascontiguousarray(eq[k * NBH : (k + 1) * NBH, :])
        in_maps.append(
            {"x": xs, "ee": ees, "eq": eqs, "ones2": ones2, "ident": ident}
        )

    nc = _get_nc()
    trace = bool(int(os.environ.get("KERNEL_TRACE", "0")))
    try:
        res = run_bass_kernel_spmd(
            nc, in_maps, core_ids=list(range(NCORES)), trace=trace
        )
    except Exception:
        # transient device state (e.g. NRT_EXEC_UNIT_UNRECOVERABLE) — retry once
        res = run_bass_kernel_spmd(
            nc, in_maps, core_ids=list(range(NCORES)), trace=trace
        )
    _CACHE["last_results"] = res

    # gather/unshard: stack per-core block maps, broadcast back onto the grid
    ub = np.concatenate(
        [res.results[k]["y"].reshape(NBH, NBW) for k in range(NCORES)], axis=0
    )  # [512, 512] per-block values
    plane = np.repeat(np.repeat(ub, BS, axis=0), BS, axis=1)  # [H, W]
    out = np.empty((B, 1, H, W), dtype=np.float32)
    out[:] = plane[None, None]
    return out


# revision 35
# speedup vs baseline: 1.2042x; 1.0232x over previous
"""Trainium2 Bass kernel for BlockUncertaintyTracker (segment_reduce), v7.

Per 4x4 block of a [16,1,2048,2048] f32 batch: block mean and 0.9-quantile
(= 0.5*(2nd+3rd largest of 16)), averaged over batch, EMA'd, ratio
broadcast back. Spatial sharding over H across 8 cores (64 block rows per
core, all 16 batches per core, no collectives). 8 groups/core of 2 batches
x 256 rows, each as 4 row-phase tiles [128=(b2,i), 2048] f32.

Key engine assignments (measured costs drove these):
  - Deinterleave+downcast: PE identity matmuls (f32r, strided rhs -- the
    16B-pitch SBUF read penalty lands on the underused tensor engine) into
    rotating PSUM banks, then cheap contiguous PSUM->SBUF f16 casts on
    ACT. Paired segment order D_r = [c0|c2|c1|c3].
  - Selection network on DVE, all step-1 contiguous f16 (2x mode):
    L1 pair max/min (4 ops [128,2048]), L2 sorted-3-of-4 per column
    (5 ops [128,2048]), L3 both column-pair merges as one contiguous op
    (7 ops [128,1024]), L4 final merge -> 2nd,3rd (6 ops [128,512]).
    NOTE: multi-dim 2-byte APs on DVE 2x mode give wrong results on HW
    (diverges from CoreSim); only plain 1-D step-1 slices are used.
  - Mean: 16 f16 matmuls/group on D segments (fp32 PSUM accumulate);
    quantile batch-fold: 2 matmuls/group on r2f/r3f (folds the + too).
  - Software pipelining: sums/qmm of group g-1 are emitted after group
    g's PE copies so the in-order PE stream never stalls the cast chain;
    EMA prescale on ACT; den/reciprocal issued before the last L3/L4.
Output: only the per-block u map [64,512] f32 per core; the host
broadcast to [B,1,H,W] is the unshard step (u is batch-independent).
"""
import os

import numpy as np

# ---- problem constants (hardcoded; kernel.py must be self-contained) ----
B = 16          # batch
H = 2048
W = 2048
BS = 4          # block size
NCORES = 8
HS = H // NCORES            # 256 rows per core
NBH = HS // BS              # 64 block rows per core
NBW = W // BS               # 512 block cols
ROWS = B * HS               # 4096 rows in a per-core slab
NGROUPS = 8                 # groups per core; each = 2 batches x 256 rows
GB = B // NGROUPS           # 2 batches per group
DECAY = 0.99
ALPHA = 0.1
EPS = 1e-5
C_MEAN = (1.0 - DECAY) / (BS * BS * B)    # fold mean-over-16-elems and batch
C_QUANT = (1.0 - DECAY) * 0.5 / B         # fold 0.5*(m2+m3) and batch mean

_CACHE = {}


def _split_multi_waits(nc):
    """This walrus build encodes at most ONE sync wait per instruction.
    Tile attaches several. Hoist excess waits onto same-engine NOPs placed
    immediately before the owning instruction (same engine stream => same
    semantics)."""
    import concourse.mybir as mybir

    plans = []  # (inst_name, extra_waits)
    for f in nc.m.functions:
        for bb in f.blocks:
            for inst in bb.instructions:
                si = getattr(inst, "sync_info", None)
                waits = list(si.on_wait) if (si and si.on_wait) else []
                if len(waits) > 1:
                    si.on_wait = [waits[-1]]
                    plans.append((inst.name, waits[:-1]))

    if not plans:
        return

    nop_for = {}
    stray = set()
    for iname, extra in plans:
        nops = []
        for w in extra:
            nop = nc.engines[nc.inst_map[iname].engine].nop(nofuse=True).ins
            nop.sync_info = mybir.SyncInfo(on_wait=[w], on_update=[])
            nops.append(nop)
            stray.add(nop.name)
        nop_for[iname] = nops

    for f in nc.m.functions:
        for bb in f.blocks:
            out = []
            changed = False
            for inst in bb.instructions:
                if inst.name in stray:
                    changed = True
                    continue
                if inst.name in nop_for:
                    out.extend(nop_for[inst.name])
                    changed = True
                out.append(inst)
            if changed:
                bb.instructions = out


def _build():
    """Builds the single-core Bass program (SPMD across 8 cores)."""
    from contextlib import ExitStack

    import concourse.bass as bass
    import concourse.mybir as mybir
    import concourse.tile as tile

    f32 = mybir.dt.float32
    f16 = mybir.dt.float16
    MAX = mybir.AluOpType.max
    MIN = mybir.AluOpType.min
    MULT = mybir.AluOpType.mult
    ADD = mybir.AluOpType.add

    nc = bass.Bass("TRN2", target_bir_lowering=False, debug=False)

    f32r = mybir.dt.float32r
    x = nc.dram_tensor("x", [ROWS, W], f32r, kind="ExternalInput").ap()
    ee = nc.dram_tensor("ee", [NBH, NBW], f32, kind="ExternalInput").ap()
    eq = nc.dram_tensor("eq", [NBH, NBW], f32, kind="ExternalInput").ap()
    # ones2[p, m] = (p % 64 == m // 2): batch-pair fold + row duplication
    ones2 = nc.dram_tensor("ones2", [128, 128], f32, kind="ExternalInput").ap()
    ident = nc.dram_tensor("ident", [128, 128], f32r, kind="ExternalInput").ap()
    # per-block uncertainty map only; host broadcasts to [B, H, W]
    y = nc.dram_tensor("y", [NBH, NBW], f32, kind="ExternalOutput").ap()

    # input: row = ((g*2 + b2)*64 + i)*4 + r; per (g, r): [128=(b2,i), 2048]
    xr = x.rearrange("(g b2 i r) w -> g r (b2 i) w", g=NGROUPS, b2=GB, i=NBH, r=BS)

    with tile.TileContext(nc) as tc, ExitStack() as ctx:
        pool = ctx.enter_context(tc.tile_pool(name="work", bufs=1))
        ppool = ctx.enter_context(tc.tile_pool(name="acc", bufs=1, space="PSUM"))

        psum_s = ppool.tile([128, NBW], f32, tag="ps")
        psum_q = ppool.tile([128, NBW], f32, tag="pq")

        ones_sb = pool.tile([128, 128], f32, tag="ones")
        nc.sync.dma_start(ones_sb[:, :], ones2)
        ones16_sb = pool.tile([128, 128], f16, tag="ones16")
        nc.scalar.copy(ones16_sb[:, :], ones_sb[:, :])
        ident_sb = pool.tile([128, 128], f32r, tag="ident")
        nc.sync.dma_start(ident_sb[:, :], ident)

        def vtt(dst, a, bb, op):
            nc.vector.tensor_tensor(dst, a, bb, op)

        rts_g = {}
        dts_g = {}
        l1_g = {}
        l2_g = {}
        l3_g = {}
        l4_g = {}
        nmm = [0]

        def emit_load(g, rs=range(BS)):
            for r in rs:
                rt = pool.tile([128, W], f32r, tag=f"r{r}", bufs=2, name=f"rt{r}_{g}")
                nc.sync.dma_start(rt[:, :], xr[g, r])
                rts_g.setdefault(g, {})[r] = rt

        def emit_cast_tile(g, r):
            # column-split via PE: identity matmul reads rt strided (the
            # 16B-pitch penalty lands on the idle tensor engine) into a
            # rotating PSUM bank; ACT then does a cheap contiguous
            # PSUM->SBUF f16 cast. D_r = [c0|c1|c2|c3] as before.
            dt = pool.tile([128, W], f16, tag=f"d{r}", bufs=2, name=f"dt{r}_{g}")
            dv = dt.rearrange("p (c j) -> p c j", c=BS)
            sv = rts_g[g][r].rearrange("p (j c) -> p j c", c=BS)
            # paired segment order [c0|c2|c1|c3]: L3's two merges read
            # contiguous 1024-wide halves (lo = c0,c2 vs hi = c1,c3)
            for seg, c in enumerate((0, 2, 1, 3)):
                pc = ppool.tile([128, NBW], f32, tag="pc", bufs=6,
                                name=f"pc_{g}_{r}_{c}")
                nc.tensor.matmul(
                    pc[:, :], lhsT=ident_sb[:, :], rhs=sv[:, :, c],
                    start=True, stop=True,
                )
                nc.scalar.copy(dv[:, seg, :], pc[:, :])
            dts_g.setdefault(g, []).append(dt)

        def emit_sum(g, tiles):
            # exact block sums: 16 f16 matmuls on contiguous 512-segments.
            # `tiles` per-position sum must equal D0+D1+D2+D3 (for the last
            # group we pass L1's max/min outputs: max+min == sum).
            for r in range(BS):
                dv = tiles[r].rearrange("p (c j) -> p c j", c=BS)
                for c in range(BS):
                    k = g * 16 + r * BS + c
                    nc.tensor.matmul(
                        psum_s[:, :], lhsT=ones16_sb[:, :], rhs=dv[:, c, :],
                        start=(k == 0), stop=(k == NGROUPS * 16 - 1),
                    )
                    nmm[0] += 1

        def emit_l1_pair(g, pair):
            # one tile-pair's max/min (pair 0 = (d0,d1), pair 1 = (d2,d3))
            a, bb = dts_g[g][2 * pair], dts_g[g][2 * pair + 1]
            for idx, op in ((2 * pair, MAX), (2 * pair + 1, MIN)):
                t = pool.tile([128, W], f16, tag="l1", bufs=8, name=f"l1_{idx}_{g}")
                vtt(t[:, :], a[:, :], bb[:, :], op)
                l1_g.setdefault(g, []).append(t)

        def emit_l2(g):
            A, Bm, C, E = l1_g[g]  # A=max01 Bm=min01 C=max23 E=min23
            # L2: per-column sorted top-3 of 4 (full width, column-split)
            r1 = pool.tile([128, W], f16, tag="l2m", bufs=6, name=f"r1_{g}")
            vtt(r1[:, :], A[:, :], C[:, :], MAX)
            xx = pool.tile([128, W], f16, tag="l2t", bufs=2, name=f"xx_{g}")
            vtt(xx[:, :], A[:, :], C[:, :], MIN)
            yy = pool.tile([128, W], f16, tag="l2t", bufs=2, name=f"yy_{g}")
            vtt(yy[:, :], Bm[:, :], E[:, :], MAX)
            r2 = pool.tile([128, W], f16, tag="l2m", bufs=6, name=f"r2_{g}")
            vtt(r2[:, :], xx[:, :], yy[:, :], MAX)
            r3 = pool.tile([128, W], f16, tag="l2m", bufs=6, name=f"r3_{g}")
            vtt(r3[:, :], xx[:, :], yy[:, :], MIN)
            l2_g[g] = (r1, r2, r3)

        def emit_l3(g):
            r1, r2, r3 = l2_g[g]
            # L3: paired layout [c0|c2|c1|c3] makes both column merges one
            # contiguous 1024-wide op: lo half = (c0,c2), hi half = (c1,c3)
            # -> the two merges (c0,c1) and (c2,c3) line up elementwise.
            HW2 = W // 2

            def mk(name):
                return pool.tile([128, HW2], f16, tag="l3", bufs=7,
                                 name=f"{name}_{g}")

            def lo(t):
                return t[:, 0:HW2]

            def hi(t):
                return t[:, HW2:W]

            s1, x2, c2, s2, mn, m3, s3 = (
                mk("s1"), mk("x2"), mk("c2"), mk("s2"), mk("mn"), mk("m3"),
                mk("s3"),
            )
            vtt(s1[:, :], lo(r1), hi(r1), MAX)   # pair rank1
            vtt(x2[:, :], lo(r1), hi(r1), MIN)
            vtt(c2[:, :], lo(r2), hi(r2), MAX)
            vtt(s2[:, :], x2[:, :], c2[:, :], MAX)  # pair rank2
            vtt(mn[:, :], x2[:, :], c2[:, :], MIN)
            vtt(m3[:, :], lo(r3), hi(r3), MAX)
            vtt(s3[:, :], mn[:, :], m3[:, :], MAX)  # pair rank3
            l3_g[g] = (s1, s2, s3)

        def emit_l4(g):
            s1, s2, s3 = l3_g[g]
            a1, b1 = s1[:, 0:NBW], s1[:, NBW : 2 * NBW]
            a2, b2 = s2[:, 0:NBW], s2[:, NBW : 2 * NBW]
            a3, b3 = s3[:, 0:NBW], s3[:, NBW : 2 * NBW]

            def op4(name, ina, inb, op):
                t = pool.tile([128, NBW], f16, tag="l4", bufs=7, name=f"{name}_{g}")
                vtt(t[:, :], ina, inb, op)
                return t

            x4 = op4("x4", a1, b1, MIN)
            c4 = op4("c4", a2, b2, MAX)
            r2f = op4("r2f", x4[:, :], c4[:, :], MAX)
            mn4 = op4("mn4", x4[:, :], c4[:, :], MIN)
            m34 = op4("m34", a3, b3, MAX)
            r3f = op4("r3f", mn4[:, :], m34[:, :], MAX)
            l4_g[g] = (r2f, r3f)

        def emit_qmm(g):
            # qs = r2f + r3f folded into PSUM accumulation: two matmuls
            r2f, r3f = l4_g[g]
            nc.tensor.matmul(
                psum_q[:, :], lhsT=ones16_sb[:, :], rhs=r2f[:, :],
                start=(g == 0), stop=False,
            )
            nc.tensor.matmul(
                psum_q[:, :], lhsT=ones16_sb[:, :], rhs=r3f[:, :],
                start=False, stop=(g == NGROUPS - 1),
            )

        def emit_ema_prescale():
            # EMA input DMAs + pre-scale; deferred past group 0 so their
            # many small broadcast descriptors don't delay the first tiles
            ee_sb = pool.tile([128, NBW], f32, tag="tail", bufs=8, name="ee_sb")
            nc.sync.dma_start(
                ee_sb[:, :], ee.unsqueeze(1).broadcast_to((NBH, 2, NBW))
            )
            eq_sb = pool.tile([128, NBW], f32, tag="tail", bufs=8, name="eq_sb")
            nc.sync.dma_start(
                eq_sb[:, :], eq.unsqueeze(1).broadcast_to((NBH, 2, NBW))
            )
            ee2 = pool.tile([128, NBW], f32, tag="tail", bufs=8, name="ee2")
            nc.scalar.activation(
                ee2[:, :], ee_sb[:, :], mybir.ActivationFunctionType.Copy,
                bias=EPS, scale=DECAY,
            )
            eq2 = pool.tile([128, NBW], f32, tag="tail", bufs=8, name="eq2")
            nc.scalar.activation(
                eq2[:, :], eq_sb[:, :], mybir.ActivationFunctionType.Copy,
                bias=0.0, scale=DECAY,
            )
            return ee2, eq2

        LAST = NGROUPS - 1
        ee2 = eq2 = None
        for g in range(NGROUPS):
            if g == 0:
                emit_load(g, rs=(0, 1))  # front-load the first two tiles
            else:
                emit_load(g)
            # interleave casts and L1 pairs so DVE starts ~halfway into the
            # cast stream; sums/qmm of the previous group are emitted AFTER
            # this group's PE copies so the in-order PE stream never delays
            # the cast chain feeding DVE
            emit_cast_tile(g, 0)
            emit_cast_tile(g, 1)
            emit_l1_pair(g, 0)
            if g == 0:
                emit_load(g, rs=(2, 3))
            emit_cast_tile(g, 2)
            emit_cast_tile(g, 3)
            if g == 1:
                ee2, eq2 = emit_ema_prescale()
            if g >= 1:
                emit_sum(g - 1, dts_g[g - 1])
                emit_qmm(g - 1)
            emit_l1_pair(g, 1)
            emit_l2(g)
            if g == LAST:
                emit_sum(g, dts_g[g])
            if g != LAST:
                emit_l3(g)
                emit_l4(g)

        # den/reciprocal depend only on psum_s (complete after last emit_sum):
        # issue them on DVE before the last group's L3/L4 to shorten the tail
        den = pool.tile([128, NBW], f32, tag="tail", bufs=8, name="den")
        nc.vector.scalar_tensor_tensor(
            den[:, :], psum_s[:, :], C_MEAN, ee2[:, :], op0=MULT, op1=ADD
        )
        rec = pool.tile([128, NBW], f32, tag="tail", bufs=8, name="rec")
        nc.vector.reciprocal(rec[:, :], den[:, :])

        emit_l3(LAST)
        emit_l4(LAST)
        emit_qmm(LAST)

        num = pool.tile([128, NBW], f32, tag="tail", bufs=8, name="num")
        nc.vector.scalar_tensor_tensor(
            num[:, :], psum_q[:, :], C_QUANT, eq2[:, :], op0=MULT, op1=ADD
        )
        u = pool.tile([128, NBW], f32, tag="tail", bufs=8, name="u")
        nc.vector.tensor_tensor(u[:, :], num[:, :], rec[:, :], MULT)

        # single small write: u[2i, j] == u[2i+1, j] == per-block value
        nc.sync.dma_start(y, u[0:128:2, :])

    _split_multi_waits(nc)
    return nc


def _get_nc():
    if "nc" not in _CACHE:
        _CACHE["nc"] = _build()
    return _CACHE["nc"]


def kernel(current_errors, ema_errors, ema_quantile):
    from concourse.bass_utils import run_bass_kernel_spmd

    x = np.asarray(current_errors, dtype=np.float32).reshape(B, H, W)
    ee = np.asarray(ema_errors, dtype=np.float32).reshape(H // BS, W // BS)
    eq = np.asarray(ema_quantile, dtype=np.float32).reshape(H // BS, W // BS)

    # ones2[p, m] == 1 iff p % 64 == m // 2
    ones2 = np.zeros((128, 128), dtype=np.float32)
    p = np.arange(128)
    ones2[p, (p % NBH) * 2] = 1.0
    ones2[p, (p % NBH) * 2 + 1] = 1.0
    ident = np.eye(128, dtype=np.float32)

    in_maps = []
    for k in range(NCORES):
        xs = np.ascontiguousarray(x[:, k * HS : (k + 1) * HS, :]).reshape(ROWS, W)
        ees = np.ascontiguousarray(ee[k * NBH : (k + 1) * NBH, :])
        eqs = np.ascontiguousarray(eq[k * NBH : (k + 1) * NBH, :])
        in_maps.append(
            {"x": xs, "ee": ees, "eq": eqs, "ones2": ones2, "ident": ident}
        )

    nc = _get_nc()
    trace = bool(int(os.environ.get("KERNEL_TRACE", "0")))
    try:
        res = run_bass_kernel_spmd(
            nc, in_maps, core_ids=list(range(NCORES)), trace=trace
        )
    except Exception:
        # transient device state (e.g. NRT_EXEC_UNIT_UNRECOVERABLE) — retry once
        res = run_bass_kernel_spmd(
            nc, in_maps, core_ids=list(range(NCORES)), trace=trace
        )
    _CACHE["last_results"] = res

    # gather/unshard: stack per-core block maps, broadcast back onto the grid
    ub = np.concatenate(
        [res.results[k]["y"].reshape(NBH, NBW) for k in range(NCORES)], axis=0
    )  # [512, 512] per-block values
    plane = np.repeat(np.repeat(ub, BS, axis=0), BS, axis=1)  # [H, W]
    out = np.empty((B, 1, H, W), dtype=np.float32)
    out[:] = plane[None, None]
    return out
